# revision 19
# baseline (speedup 1.0000x reference)
"""YOLOv1 loss kernel for Trainium2, 8-core data-parallel.

Strategy: shard batch (8192) across 8 cores (1024 each). Each core
processes its shard in NCHUNK chunks of G*128 batch items laid out as
[128 partitions, G, 1470] in SBUF (channel-major free dim). All box/IoU
arithmetic runs on the Vector engine (fused scalar_tensor_tensor ops
where possible); sqrt/square run on the Scalar (ACT) engine; DMAs on the
Sync (HWDGE) engine. Per-partition partial sums accumulate on-chip via
tensor_tensor_reduce; host sums the 8x128x3 partials and divides by B.

IoU is computed in cell-relative scaled coordinates: all three boxes of
a cell share the same (+m, +n)/G offset, so IoU is invariant to it, and
invariant to a uniform x-scale. With half-extents h = 3.5*w the overlap
width is min(2*ha, 2*hb, ha+hb-|dcx|) clamped at 0 (units: 7*w), and
union = 49*(wa*ha_frac...) i.e. 49*(w_a*h_a + w_g*h_g) - inter.
"""

import sys

import numpy as np

for _p in ("/opt/trn_rl_repo", "/root/.axon_site/_ro/trn_rl_repo"):
    if _p not in sys.path:
        sys.path.insert(0, _p)

import concourse.bass as bass
import concourse.mybir as mybir
from concourse.bass_utils import run_bass_kernel_spmd

F32 = mybir.dt.float32
U32 = mybir.dt.uint32
Alu = mybir.AluOpType
Act = mybir.ActivationFunctionType

B_TOTAL = 8192
NCORES = 8
B_CORE = B_TOTAL // NCORES  # 1024
P = 128
G = 2  # batch groups folded into the free dim per chunk
CHUNK = P * G  # 256
NCHUNK = B_CORE // CHUNK  # 4
C = 30
CELLS = 49
ROW = C * CELLS  # 1470


def build_nc(g: int = G, nchunk: int = NCHUNK):
    chunk = P * g
    nc = bass.Bass()
    pred = nc.declare_dram_parameter("pred", [B_CORE, ROW], F32, isOutput=False)
    labels = nc.declare_dram_parameter("labels", [B_CORE, ROW], F32, isOutput=False)
    out = nc.declare_dram_parameter("out", [P, 4], F32, isOutput=True)

    fshape = [P, g, CELLS]
    bshape = [P, g, 20 * CELLS]

    _ctr = [0]

    def sb(shape):
        _ctr[0] += 1
        return ctx_stack.enter_context(
            nc.sbuf_tensor(f"t{_ctr[0]}", shape, F32)
        )

    from contextlib import ExitStack

    ctx_stack = ExitStack()
    with ctx_stack:
        # double-buffered input tiles
        pt = [sb([P, g, ROW]) for _ in range(2)]
        lt = [sb([P, g, ROW]) for _ in range(2)]
        # ACT outputs (single-buffered; protected by sem schedule)
        sp2, sl2, sp3, sl3 = sb(fshape), sb(fshape), sb(fshape), sb(fshape)
        sp7, sl7, sp8, sl8 = sb(fshape), sb(fshape), sb(fshape), sb(fshape)
        q4, q9 = sb(fshape), sb(fshape)
        qx1, qy1, qx2, qy2 = sb(fshape), sb(fshape), sb(fshape), sb(fshape)
        e1, e2 = sb(fshape), sb(fshape)
        qsw1, qsh1, qsw2, qsh2 = sb(fshape), sb(fshape), sb(fshape), sb(fshape)
        sqcls = sb(bshape)
        # DVE temps
        dx1, dy1, dx2, dy2 = sb(fshape), sb(fshape), sb(fshape), sb(fshape)
        dx2c, dy2c = sb(fshape), sb(fshape)
        adx1, ady1, adx2, ady2 = sb(fshape), sb(fshape), sb(fshape), sb(fshape)
        sw1, sh1, sw2, sh2 = sb(fshape), sb(fshape), sb(fshape), sb(fshape)
        ta1, tb1, tc1 = sb(fshape), sb(fshape), sb(fshape)
        ta2, tb2, tc2 = sb(fshape), sb(fshape), sb(fshape)
        ta3, tb3, tc3 = sb(fshape), sb(fshape), sb(fshape)
        ta4, tb4, tc4 = sb(fshape), sb(fshape), sb(fshape)
        iw1, ih1, iw2, ih2 = sb(fshape), sb(fshape), sb(fshape), sb(fshape)
        int1, int2 = sb(fshape), sb(fshape)
        a1, a2, ag = sb(fshape), sb(fshape), sb(fshape)
        s1, s2 = sb(fshape), sb(fshape)
        u1, u2 = sb(fshape), sb(fshape)
        r1, r2 = sb(fshape), sb(fshape)
        iou1, iou2 = sb(fshape), sb(fshape)
        use1, objm = sb(fshape), sb(fshape)
        d1, d2 = sb(fshape), sb(fshape)
        dcls = sb(bshape)
        jbig = sb(bshape)
        dsw1, dsh1, dsw2, dsh2 = sb(fshape), sb(fshape), sb(fshape), sb(fshape)
        c1a, c1b, c2a, c2b = sb(fshape), sb(fshape), sb(fshape), sb(fshape)
        coor1, coor2 = sb(fshape), sb(fshape)
        de, hde, dc = sb(fshape), sb(fshape), sb(fshape)
        nc2t, mix, tsel = sb(fshape), sb(fshape), sb(fshape)
        clsum = sb(fshape)
        base, base2, base3, dd = sb(fshape), sb(fshape), sb(fshape), sb(fshape)
        junk = sb(fshape)
        junk2 = sb(fshape)
        red0 = ctx_stack.enter_context(nc.sbuf_tensor("red0", [P, 1], F32))
        red1 = ctx_stack.enter_context(nc.sbuf_tensor("red1", [P, 1], F32))
        acc = ctx_stack.enter_context(nc.sbuf_tensor("acc", [P, 4], F32))

        dma_sem = ctx_stack.enter_context(nc.semaphore("dma_sem"))
        sA1 = ctx_stack.enter_context(nc.semaphore("sA1"))
        sA2 = ctx_stack.enter_context(nc.semaphore("sA2"))
        sD1 = ctx_stack.enter_context(nc.semaphore("sD1"))
        sD2 = ctx_stack.enter_context(nc.semaphore("sD2"))
        v_done = ctx_stack.enter_context(nc.semaphore("v_done"))
        block = ctx_stack.enter_context(nc.Block())

        def ch(t, c):  # channel slice -> [P, g, 49]
            return t[:, :, c * CELLS:(c + 1) * CELLS]

        def cls_blk(t):  # channels 10..29 -> [P, g, 980]
            return t[:, :, 10 * CELLS:30 * CELLS]

        @block.sync
        def _(sync):
            for i in range(nchunk):
                s = i % 2
                if i >= 1:
                    sync.wait_ge(dma_sem, 32 * i)
                if i >= 2:
                    sync.wait_ge(v_done, i - 1)
                rows = slice(i * chunk, (i + 1) * chunk)
                sync.dma_start(
                    out=pt[s][:],
                    in_=pred[rows].rearrange("(g p) d -> p g d", p=P),
                ).then_inc(dma_sem, 16)
                sync.dma_start(
                    out=lt[s][:],
                    in_=labels[rows].rearrange("(g p) d -> p g d", p=P),
                ).then_inc(dma_sem, 16)
            sync.wait_ge(v_done, nchunk)
            sync.dma_start(out=out[:], in_=acc[:]).then_inc(dma_sem, 16)
            sync.wait_ge(dma_sem, 32 * nchunk + 16)

        @block.scalar
        def _(act):
            for i in range(nchunk):
                s = i % 2
                if i >= 1:
                    act.wait_ge(v_done, i)
                act.wait_ge(dma_sem, 32 * (i + 1))
                p, l = pt[s], lt[s]
                # phase 1: sqrts of w/h channels + conf squares
                act.activation(sp2[:], ch(p, 2), Act.Sqrt)
                act.activation(sl2[:], ch(l, 2), Act.Sqrt)
                act.activation(sp3[:], ch(p, 3), Act.Sqrt)
                act.activation(sl3[:], ch(l, 3), Act.Sqrt)
                act.activation(sp7[:], ch(p, 7), Act.Sqrt)
                act.activation(sl7[:], ch(l, 7), Act.Sqrt)
                act.activation(sp8[:], ch(p, 8), Act.Sqrt)
                act.activation(sl8[:], ch(l, 8), Act.Sqrt)
                act.activation(q4[:], ch(p, 4), Act.Square)
                act.activation(q9[:], ch(p, 9), Act.Square)
                act.drain().then_inc(sA1, 1)
                # phase 2a: squares of DVE diffs
                act.wait_ge(sD1, i + 1)
                act.activation(qx1[:], dx1[:], Act.Square)
                act.activation(qy1[:], dy1[:], Act.Square)
                act.activation(qx2[:], dx2c[:], Act.Square)
                act.activation(qy2[:], dy2c[:], Act.Square)
                act.activation(e1[:], d1[:], Act.Square)
                act.activation(e2[:], d2[:], Act.Square)
                act.activation(sqcls[:], dcls[:], Act.Square)
                # phase 2b: squares of sqrt diffs
                act.wait_ge(sD2, i + 1)
                act.activation(qsw1[:], dsw1[:], Act.Square)
                act.activation(qsh1[:], dsh1[:], Act.Square)
                act.activation(qsw2[:], dsw2[:], Act.Square)
                act.activation(qsh2[:], dsh2[:], Act.Square)
                act.drain().then_inc(sA2, 1)

        @block.vector
        def _(v):
            stt = v.scalar_tensor_tensor
            tt = v.tensor_tensor
            ts = v.tensor_scalar

            v.memset(acc[:], 0.0)
            v.drain()
            for i in range(nchunk):
                s = i % 2
                v.wait_ge(dma_sem, 32 * (i + 1))
                p, l = pt[s], lt[s]
                # --- wave 1: direct from inputs ---
                tt(dx1[:], ch(p, 0), ch(l, 0), Alu.subtract)
                tt(dy1[:], ch(p, 1), ch(l, 1), Alu.subtract)
                tt(dx2[:], ch(p, 5), ch(l, 0), Alu.subtract)
                tt(dy2[:], ch(p, 6), ch(l, 1), Alu.subtract)
                tt(dx2c[:], ch(p, 5), ch(l, 5), Alu.subtract)
                tt(dy2c[:], ch(p, 6), ch(l, 6), Alu.subtract)
                tt(sw1[:], ch(p, 2), ch(l, 2), Alu.add)
                tt(sh1[:], ch(p, 3), ch(l, 3), Alu.add)
                tt(sw2[:], ch(p, 7), ch(l, 2), Alu.add)
                tt(sh2[:], ch(p, 8), ch(l, 3), Alu.add)
                tt(a1[:], ch(p, 2), ch(p, 3), Alu.mult)
                tt(a2[:], ch(p, 7), ch(p, 8), Alu.mult)
                tt(ag[:], ch(l, 2), ch(l, 3), Alu.mult)
                tt(dcls[:], cls_blk(p), cls_blk(l), Alu.subtract)
                ts(objm[:], ch(l, 4), 1.0, None, Alu.is_equal)
                v.drain()
                # --- wave 2 ---
                ts(adx1[:].bitcast(U32), dx1[:].bitcast(U32), 0x7FFFFFFF, None,
                   Alu.bitwise_and)
                ts(ady1[:].bitcast(U32), dy1[:].bitcast(U32), 0x7FFFFFFF, None,
                   Alu.bitwise_and)
                ts(adx2[:].bitcast(U32), dx2[:].bitcast(U32), 0x7FFFFFFF, None,
                   Alu.bitwise_and)
                ts(ady2[:].bitcast(U32), dy2[:].bitcast(U32), 0x7FFFFFFF, None,
                   Alu.bitwise_and)
                tt(s1[:], a1[:], ag[:], Alu.add)
                tt(s2[:], a2[:], ag[:], Alu.add)
                v.drain()
                # --- wave 3: overlap = min(S-|d|, 7wa, 7wb), clamped ---
                stt(ta1[:], sw1[:], 3.5, adx1[:], Alu.mult, Alu.subtract)
                stt(ta2[:], sh1[:], 3.5, ady1[:], Alu.mult, Alu.subtract)
                stt(ta3[:], sw2[:], 3.5, adx2[:], Alu.mult, Alu.subtract)
                stt(ta4[:], sh2[:], 3.5, ady2[:], Alu.mult, Alu.subtract)
                v.drain()
                # --- wave 4 ---
                stt(tb1[:], ch(p, 2), 7.0, ta1[:], Alu.mult, Alu.min)
                stt(tb2[:], ch(p, 3), 7.0, ta2[:], Alu.mult, Alu.min)
                stt(tb3[:], ch(p, 7), 7.0, ta3[:], Alu.mult, Alu.min)
                stt(tb4[:], ch(p, 8), 7.0, ta4[:], Alu.mult, Alu.min)
                v.drain()
                # --- wave 5 ---
                stt(tc1[:], ch(l, 2), 7.0, tb1[:], Alu.mult, Alu.min)
                stt(tc2[:], ch(l, 3), 7.0, tb2[:], Alu.mult, Alu.min)
                stt(tc3[:], ch(l, 2), 7.0, tb3[:], Alu.mult, Alu.min)
                stt(tc4[:], ch(l, 3), 7.0, tb4[:], Alu.mult, Alu.min)
                v.drain()
                # --- wave 6: clamp ---
                ts(iw1[:], tc1[:], 0.0, None, Alu.max)
                ts(ih1[:], tc2[:], 0.0, None, Alu.max)
                ts(iw2[:], tc3[:], 0.0, None, Alu.max)
                ts(ih2[:], tc4[:], 0.0, None, Alu.max)
                v.drain()
                # --- wave 7 ---
                tt(int1[:], iw1[:], ih1[:], Alu.mult)
                tt(int2[:], iw2[:], ih2[:], Alu.mult)
                v.drain()
                # --- wave 8: union = 49*(area_p + area_g) - inter ---
                stt(u1[:], s1[:], 49.0, int1[:], Alu.mult, Alu.subtract)
                stt(u2[:], s2[:], 49.0, int2[:], Alu.mult, Alu.subtract)
                v.drain()
                # --- wave 9 ---
                v.reciprocal(r1[:], u1[:])
                v.reciprocal(r2[:], u2[:])
                v.drain()
                # --- wave 10 ---
                tt(iou1[:], int1[:], r1[:], Alu.mult)
                tt(iou2[:], int2[:], r2[:], Alu.mult)
                v.drain()
                # --- wave 11 ---
                tt(use1[:], iou1[:], iou2[:], Alu.is_ge)
                tt(d1[:], ch(p, 4), iou1[:], Alu.subtract)
                tt(d2[:], ch(p, 9), iou2[:], Alu.subtract)
                v.drain().then_inc(sD1, 1)
                # --- wave 12: sqrt diffs (needs ACT phase 1) ---
                v.wait_ge(sA1, i + 1)
                tt(dsw1[:], sp2[:], sl2[:], Alu.subtract)
                tt(dsh1[:], sp3[:], sl3[:], Alu.subtract)
                tt(dsw2[:], sp7[:], sl7[:], Alu.subtract)
                tt(dsh2[:], sp8[:], sl8[:], Alu.subtract)
                v.drain().then_inc(sD2, 1)
                # --- wave 13+: combine (needs ACT phase 2) ---
                v.wait_ge(sA2, i + 1)
                v.tensor_reduce(
                    out=clsum[:],
                    in_=sqcls[:].rearrange("p g (c k) -> p g k c", c=20),
                    axis=mybir.AxisListType.X, op=Alu.add,
                )
                tt(de[:], e1[:], e2[:], Alu.subtract)
                tt(nc2t[:], q4[:], q9[:], Alu.add)
                tt(c1a[:], qx1[:], qy1[:], Alu.add)
                tt(c1b[:], qsw1[:], qsh1[:], Alu.add)
                tt(c2a[:], qx2[:], qy2[:], Alu.add)
                tt(c2b[:], qsw2[:], qsh2[:], Alu.add)
                v.drain()
                tt(coor1[:], c1a[:], c1b[:], Alu.add)
                tt(coor2[:], c2a[:], c2b[:], Alu.add)
                ts(hde[:], de[:], 0.5, None, Alu.mult)
                v.drain()
                tt(dc[:], coor1[:], coor2[:], Alu.subtract)
                stt(base[:], coor2[:], 5.0, e2[:], Alu.mult, Alu.add)
                v.drain()
                stt(mix[:], dc[:], 5.0, hde[:], Alu.mult, Alu.add)
                stt(base2[:], e1[:], 0.5, base[:], Alu.mult, Alu.add)
                v.drain()
                tt(tsel[:], use1[:], mix[:], Alu.mult)
                tt(junk2[:], clsum[:], base2[:], Alu.add)
                v.drain()
                tt(base3[:], junk2[:], tsel[:], Alu.add)
                v.drain()
                stt(dd[:], nc2t[:], -0.5, base3[:], Alu.mult, Alu.add)
                v.drain()
                # accumulate: acc0 += sum(obj * dd); acc1 += 0.5*sum(nc2)
                tt(junk[:], objm[:], dd[:], Alu.mult)
                v.drain()
                v.tensor_reduce(out=red0[:], in_=junk[:],
                                axis=mybir.AxisListType.XY, op=Alu.add)
                v.tensor_reduce(out=red1[:], in_=nc2t[:],
                                axis=mybir.AxisListType.XY, op=Alu.add)
                v.drain()
                stt(acc[:, 0:1], red0[:], 1.0, acc[:, 0:1], Alu.mult, Alu.add)
                stt(acc[:, 1:2], red1[:], 0.5, acc[:, 1:2], Alu.mult, Alu.add)
                v.drain().then_inc(v_done, 1)

    return nc


_NC_CACHE = {}


def _get_nc():
    if "nc" not in _NC_CACHE:
        _NC_CACHE["nc"] = build_nc()
    return _NC_CACHE["nc"]


def run_device(pred, labels, trace=False):
    nc = _get_nc()
    pred = np.ascontiguousarray(pred, dtype=np.float32).reshape(B_TOTAL, ROW)
    labels = np.ascontiguousarray(labels, dtype=np.float32).reshape(B_TOTAL, ROW)
    in_maps = []
    for c in range(NCORES):
        rows = slice(c * B_CORE, (c + 1) * B_CORE)
        in_maps.append({"pred": pred[rows], "labels": labels[rows]})
    res = run_bass_kernel_spmd(nc, in_maps, list(range(NCORES)), trace=trace)
    total = 0.0
    for c in range(NCORES):
        total += float(res.results[c]["out"][:, :3].astype(np.float64).sum())
    loss = np.float32(total / B_TOTAL)
    return loss, res


def kernel(pred, labels):
    loss, _ = run_device(pred, labels, trace=False)
    return np.array(loss, dtype=np.float32)


if __name__ == "__main__":
    rng = np.random.default_rng(0)
    p = rng.random((B_TOTAL, C, 7, 7), dtype=np.float32)
    l = rng.random((B_TOTAL, C, 7, 7), dtype=np.float32)
    l[:, 4] = (rng.random((B_TOTAL, 7, 7)) < 0.3).astype(np.float32)
    print(kernel(p, l))


# revision 20
# speedup vs baseline: 1.0029x; 1.0029x over previous
"""YOLOv1 loss kernel for Trainium2, 8-core data-parallel.

Strategy: shard batch (8192) across 8 cores (1024 each). Each core
processes its shard in NCHUNK chunks of G*128 batch items laid out as
[128 partitions, G, 1470] in SBUF (channel-major free dim). All box/IoU
arithmetic runs on the Vector engine (fused scalar_tensor_tensor ops
where possible); sqrt/square run on the Scalar (ACT) engine; DMAs on the
Sync (HWDGE) engine. Per-partition partial sums accumulate on-chip via
tensor_tensor_reduce; host sums the 8x128x3 partials and divides by B.

IoU is computed in cell-relative scaled coordinates: all three boxes of
a cell share the same (+m, +n)/G offset, so IoU is invariant to it, and
invariant to a uniform x-scale. With half-extents h = 3.5*w the overlap
width is min(2*ha, 2*hb, ha+hb-|dcx|) clamped at 0 (units: 7*w), and
union = 49*(wa*ha_frac...) i.e. 49*(w_a*h_a + w_g*h_g) - inter.
"""

import sys

import numpy as np

for _p in ("/opt/trn_rl_repo", "/root/.axon_site/_ro/trn_rl_repo"):
    if _p not in sys.path:
        sys.path.insert(0, _p)

import concourse.bass as bass
import concourse.mybir as mybir
from concourse.bass_utils import run_bass_kernel_spmd

F32 = mybir.dt.float32
U32 = mybir.dt.uint32
Alu = mybir.AluOpType
Act = mybir.ActivationFunctionType

B_TOTAL = 8192
NCORES = 8
B_CORE = B_TOTAL // NCORES  # 1024
P = 128
G = 2  # batch groups folded into the free dim per chunk
CHUNK = P * G  # 256
NCHUNK = B_CORE // CHUNK  # 4
C = 30
CELLS = 49
ROW = C * CELLS  # 1470


def build_nc(g: int = G, nchunk: int = NCHUNK):
    chunk = P * g
    nc = bass.Bass()
    pred = nc.declare_dram_parameter("pred", [B_CORE, ROW], F32, isOutput=False)
    labels = nc.declare_dram_parameter("labels", [B_CORE, ROW], F32, isOutput=False)
    out = nc.declare_dram_parameter("out", [P, 4], F32, isOutput=True)

    fshape = [P, g, CELLS]
    bshape = [P, g, 20 * CELLS]

    _ctr = [0]

    def sb(shape):
        _ctr[0] += 1
        return ctx_stack.enter_context(
            nc.sbuf_tensor(f"t{_ctr[0]}", shape, F32)
        )

    from contextlib import ExitStack

    ctx_stack = ExitStack()
    with ctx_stack:
        # double-buffered input tiles
        pt = [sb([P, g, ROW]) for _ in range(2)]
        lt = [sb([P, g, ROW]) for _ in range(2)]
        # ACT outputs (single-buffered; protected by sem schedule)
        sp2, sl2, sp3, sl3 = sb(fshape), sb(fshape), sb(fshape), sb(fshape)
        sp7, sl7, sp8, sl8 = sb(fshape), sb(fshape), sb(fshape), sb(fshape)
        q4, q9 = sb(fshape), sb(fshape)
        qx1, qy1, qx2, qy2 = sb(fshape), sb(fshape), sb(fshape), sb(fshape)
        e1, e2 = sb(fshape), sb(fshape)
        qsw1, qsh1, qsw2, qsh2 = sb(fshape), sb(fshape), sb(fshape), sb(fshape)
        sqcls = sb(bshape)
        # DVE temps
        dx1, dy1, dx2, dy2 = sb(fshape), sb(fshape), sb(fshape), sb(fshape)
        dx2c, dy2c = sb(fshape), sb(fshape)
        adx1, ady1, adx2, ady2 = sb(fshape), sb(fshape), sb(fshape), sb(fshape)
        sw1, sh1, sw2, sh2 = sb(fshape), sb(fshape), sb(fshape), sb(fshape)
        ta1, tb1, tc1 = sb(fshape), sb(fshape), sb(fshape)
        ta2, tb2, tc2 = sb(fshape), sb(fshape), sb(fshape)
        ta3, tb3, tc3 = sb(fshape), sb(fshape), sb(fshape)
        ta4, tb4, tc4 = sb(fshape), sb(fshape), sb(fshape)
        iw1, ih1, iw2, ih2 = sb(fshape), sb(fshape), sb(fshape), sb(fshape)
        int1, int2 = sb(fshape), sb(fshape)
        a1, a2, ag = sb(fshape), sb(fshape), sb(fshape)
        s1, s2 = sb(fshape), sb(fshape)
        u1, u2 = sb(fshape), sb(fshape)
        r1, r2 = sb(fshape), sb(fshape)
        iou1, iou2 = sb(fshape), sb(fshape)
        use1, objm = sb(fshape), sb(fshape)
        d1, d2 = sb(fshape), sb(fshape)
        dcls = sb(bshape)
        jbig = sb(bshape)
        dsw1, dsh1, dsw2, dsh2 = sb(fshape), sb(fshape), sb(fshape), sb(fshape)
        c1a, c1b, c2a, c2b = sb(fshape), sb(fshape), sb(fshape), sb(fshape)
        coor1, coor2 = sb(fshape), sb(fshape)
        de, hde, dc = sb(fshape), sb(fshape), sb(fshape)
        nc2t, mix, tsel = sb(fshape), sb(fshape), sb(fshape)
        clsum = sb(fshape)
        base, base2, base3, dd = sb(fshape), sb(fshape), sb(fshape), sb(fshape)
        junk = sb(fshape)
        junk2 = sb(fshape)
        red0 = ctx_stack.enter_context(nc.sbuf_tensor("red0", [P, 1], F32))
        red1 = ctx_stack.enter_context(nc.sbuf_tensor("red1", [P, 1], F32))
        acc = ctx_stack.enter_context(nc.sbuf_tensor("acc", [P, 4], F32))

        dma_sem = ctx_stack.enter_context(nc.semaphore("dma_sem"))
        sA1 = ctx_stack.enter_context(nc.semaphore("sA1"))
        sA2 = ctx_stack.enter_context(nc.semaphore("sA2"))
        sD1 = ctx_stack.enter_context(nc.semaphore("sD1"))
        sD2 = ctx_stack.enter_context(nc.semaphore("sD2"))
        v_done = ctx_stack.enter_context(nc.semaphore("v_done"))
        block = ctx_stack.enter_context(nc.Block())

        def ch(t, c):  # channel slice -> [P, g, 49]
            return t[:, :, c * CELLS:(c + 1) * CELLS]

        def cls_blk(t):  # channels 10..29 -> [P, g, 980]
            return t[:, :, 10 * CELLS:30 * CELLS]

        @block.sync
        def _(sync):
            for i in range(nchunk):
                s = i % 2
                if i >= 1:
                    sync.wait_ge(dma_sem, 32 * i)
                if i >= 2:
                    sync.wait_ge(v_done, i - 1)
                rows = slice(i * chunk, (i + 1) * chunk)
                sync.dma_start(
                    out=pt[s][:],
                    in_=pred[rows].rearrange("(g p) d -> p g d", p=P),
                ).then_inc(dma_sem, 16)
                sync.dma_start(
                    out=lt[s][:],
                    in_=labels[rows].rearrange("(g p) d -> p g d", p=P),
                ).then_inc(dma_sem, 16)
            sync.wait_ge(v_done, nchunk)
            sync.dma_start(out=out[:], in_=acc[:]).then_inc(dma_sem, 16)
            sync.wait_ge(dma_sem, 32 * nchunk + 16)

        @block.scalar
        def _(act):
            for i in range(nchunk):
                s = i % 2
                if i >= 1:
                    act.wait_ge(v_done, i)
                act.wait_ge(dma_sem, 32 * (i + 1))
                p, l = pt[s], lt[s]
                # phase 1: sqrts of w/h channels + conf squares
                act.activation(sp2[:], ch(p, 2), Act.Sqrt)
                act.activation(sl2[:], ch(l, 2), Act.Sqrt)
                act.activation(sp3[:], ch(p, 3), Act.Sqrt)
                act.activation(sl3[:], ch(l, 3), Act.Sqrt)
                act.activation(sp7[:], ch(p, 7), Act.Sqrt)
                act.activation(sl7[:], ch(l, 7), Act.Sqrt)
                act.activation(sp8[:], ch(p, 8), Act.Sqrt)
                act.activation(sl8[:], ch(l, 8), Act.Sqrt)
                act.activation(q4[:], ch(p, 4), Act.Square)
                act.activation(q9[:], ch(p, 9), Act.Square)
                act.drain().then_inc(sA1, 1)
                # phase 2a: squares of DVE diffs
                act.wait_ge(sD1, i + 1)
                act.activation(qx1[:], dx1[:], Act.Square)
                act.activation(qy1[:], dy1[:], Act.Square)
                act.activation(qx2[:], dx2c[:], Act.Square)
                act.activation(qy2[:], dy2c[:], Act.Square)
                act.activation(e1[:], d1[:], Act.Square)
                act.activation(e2[:], d2[:], Act.Square)
                act.activation(sqcls[:], dcls[:], Act.Square)
                # phase 2b: squares of sqrt diffs
                act.wait_ge(sD2, i + 1)
                act.activation(qsw1[:], dsw1[:], Act.Square)
                act.activation(qsh1[:], dsh1[:], Act.Square)
                act.activation(qsw2[:], dsw2[:], Act.Square)
                act.activation(qsh2[:], dsh2[:], Act.Square)
                act.drain().then_inc(sA2, 1)

        @block.vector
        def _(v):
            stt = v.scalar_tensor_tensor
            tt = v.tensor_tensor
            ts = v.tensor_scalar

            v.memset(acc[:], 0.0)
            v.drain()
            for i in range(nchunk):
                s = i % 2
                v.wait_ge(dma_sem, 32 * (i + 1))
                p, l = pt[s], lt[s]
                # --- wave 1: direct from inputs ---
                tt(dx1[:], ch(p, 0), ch(l, 0), Alu.subtract)
                tt(dy1[:], ch(p, 1), ch(l, 1), Alu.subtract)
                tt(dx2[:], ch(p, 5), ch(l, 0), Alu.subtract)
                tt(dy2[:], ch(p, 6), ch(l, 1), Alu.subtract)
                tt(dx2c[:], ch(p, 5), ch(l, 5), Alu.subtract)
                tt(dy2c[:], ch(p, 6), ch(l, 6), Alu.subtract)
                tt(sw1[:], ch(p, 2), ch(l, 2), Alu.add)
                tt(sh1[:], ch(p, 3), ch(l, 3), Alu.add)
                tt(sw2[:], ch(p, 7), ch(l, 2), Alu.add)
                tt(sh2[:], ch(p, 8), ch(l, 3), Alu.add)
                tt(tc1[:], ch(p, 2), ch(l, 2), Alu.min)
                tt(tc2[:], ch(p, 3), ch(l, 3), Alu.min)
                tt(tc3[:], ch(p, 7), ch(l, 2), Alu.min)
                tt(tc4[:], ch(p, 8), ch(l, 3), Alu.min)
                tt(a1[:], ch(p, 2), ch(p, 3), Alu.mult)
                tt(a2[:], ch(p, 7), ch(p, 8), Alu.mult)
                tt(ag[:], ch(l, 2), ch(l, 3), Alu.mult)
                tt(dcls[:], cls_blk(p), cls_blk(l), Alu.subtract)
                ts(objm[:], ch(l, 4), 1.0, None, Alu.is_equal)
                v.drain()
                # --- wave 2 ---
                ts(adx1[:].bitcast(U32), dx1[:].bitcast(U32), 0x7FFFFFFF, None,
                   Alu.bitwise_and)
                ts(ady1[:].bitcast(U32), dy1[:].bitcast(U32), 0x7FFFFFFF, None,
                   Alu.bitwise_and)
                ts(adx2[:].bitcast(U32), dx2[:].bitcast(U32), 0x7FFFFFFF, None,
                   Alu.bitwise_and)
                ts(ady2[:].bitcast(U32), dy2[:].bitcast(U32), 0x7FFFFFFF, None,
                   Alu.bitwise_and)
                tt(s1[:], a1[:], ag[:], Alu.add)
                tt(s2[:], a2[:], ag[:], Alu.add)
                v.drain()
                # --- wave 3: overlap = min(S-|d|, 7wa, 7wb), clamped ---
                stt(ta1[:], sw1[:], 3.5, adx1[:], Alu.mult, Alu.subtract)
                stt(ta2[:], sh1[:], 3.5, ady1[:], Alu.mult, Alu.subtract)
                stt(ta3[:], sw2[:], 3.5, adx2[:], Alu.mult, Alu.subtract)
                stt(ta4[:], sh2[:], 3.5, ady2[:], Alu.mult, Alu.subtract)
                v.drain()
                # --- wave 4: min vs 7*min(wa,wb) ---
                stt(tb1[:], tc1[:], 7.0, ta1[:], Alu.mult, Alu.min)
                stt(tb2[:], tc2[:], 7.0, ta2[:], Alu.mult, Alu.min)
                stt(tb3[:], tc3[:], 7.0, ta3[:], Alu.mult, Alu.min)
                stt(tb4[:], tc4[:], 7.0, ta4[:], Alu.mult, Alu.min)
                v.drain()
                # --- wave 5: clamp ---
                ts(iw1[:], tb1[:], 0.0, None, Alu.max)
                ts(ih1[:], tb2[:], 0.0, None, Alu.max)
                ts(iw2[:], tb3[:], 0.0, None, Alu.max)
                ts(ih2[:], tb4[:], 0.0, None, Alu.max)
                v.drain()
                # --- wave 7 ---
                tt(int1[:], iw1[:], ih1[:], Alu.mult)
                tt(int2[:], iw2[:], ih2[:], Alu.mult)
                v.drain()
                # --- wave 8: union = 49*(area_p + area_g) - inter ---
                stt(u1[:], s1[:], 49.0, int1[:], Alu.mult, Alu.subtract)
                stt(u2[:], s2[:], 49.0, int2[:], Alu.mult, Alu.subtract)
                v.drain()
                # --- wave 9 ---
                v.reciprocal(r1[:], u1[:])
                v.reciprocal(r2[:], u2[:])
                v.drain()
                # --- wave 10 ---
                tt(iou1[:], int1[:], r1[:], Alu.mult)
                tt(iou2[:], int2[:], r2[:], Alu.mult)
                v.drain()
                # --- wave 11 ---
                tt(use1[:], iou1[:], iou2[:], Alu.is_ge)
                tt(d1[:], ch(p, 4), iou1[:], Alu.subtract)
                tt(d2[:], ch(p, 9), iou2[:], Alu.subtract)
                v.drain().then_inc(sD1, 1)
                # --- wave 12: sqrt diffs (needs ACT phase 1) ---
                v.wait_ge(sA1, i + 1)
                tt(dsw1[:], sp2[:], sl2[:], Alu.subtract)
                tt(dsh1[:], sp3[:], sl3[:], Alu.subtract)
                tt(dsw2[:], sp7[:], sl7[:], Alu.subtract)
                tt(dsh2[:], sp8[:], sl8[:], Alu.subtract)
                v.drain().then_inc(sD2, 1)
                # --- wave 13+: combine (needs ACT phase 2) ---
                v.wait_ge(sA2, i + 1)
                v.tensor_reduce(
                    out=clsum[:],
                    in_=sqcls[:].rearrange("p g (c k) -> p g k c", c=20),
                    axis=mybir.AxisListType.X, op=Alu.add,
                )
                tt(de[:], e1[:], e2[:], Alu.subtract)
                tt(nc2t[:], q4[:], q9[:], Alu.add)
                tt(c1a[:], qx1[:], qy1[:], Alu.add)
                tt(c1b[:], qsw1[:], qsh1[:], Alu.add)
                tt(c2a[:], qx2[:], qy2[:], Alu.add)
                tt(c2b[:], qsw2[:], qsh2[:], Alu.add)
                v.drain()
                tt(coor1[:], c1a[:], c1b[:], Alu.add)
                tt(coor2[:], c2a[:], c2b[:], Alu.add)
                ts(hde[:], de[:], 0.5, None, Alu.mult)
                v.drain()
                tt(dc[:], coor1[:], coor2[:], Alu.subtract)
                stt(base[:], coor2[:], 5.0, e2[:], Alu.mult, Alu.add)
                v.drain()
                stt(mix[:], dc[:], 5.0, hde[:], Alu.mult, Alu.add)
                stt(base2[:], e1[:], 0.5, base[:], Alu.mult, Alu.add)
                v.drain()
                tt(tsel[:], use1[:], mix[:], Alu.mult)
                tt(junk2[:], clsum[:], base2[:], Alu.add)
                v.drain()
                tt(base3[:], junk2[:], tsel[:], Alu.add)
                v.drain()
                stt(dd[:], nc2t[:], -0.5, base3[:], Alu.mult, Alu.add)
                v.drain()
                # accumulate: acc0 += sum(obj * dd); acc1 += 0.5*sum(nc2)
                tt(junk[:], objm[:], dd[:], Alu.mult)
                v.drain()
                v.tensor_reduce(out=red0[:], in_=junk[:],
                                axis=mybir.AxisListType.XY, op=Alu.add)
                v.tensor_reduce(out=red1[:], in_=nc2t[:],
                                axis=mybir.AxisListType.XY, op=Alu.add)
                v.drain()
                stt(acc[:, 0:1], red0[:], 1.0, acc[:, 0:1], Alu.mult, Alu.add)
                stt(acc[:, 1:2], red1[:], 0.5, acc[:, 1:2], Alu.mult, Alu.add)
                v.drain().then_inc(v_done, 1)

    return nc


_NC_CACHE = {}


def _get_nc():
    if "nc" not in _NC_CACHE:
        _NC_CACHE["nc"] = build_nc()
    return _NC_CACHE["nc"]


def run_device(pred, labels, trace=False):
    nc = _get_nc()
    pred = np.ascontiguousarray(pred, dtype=np.float32).reshape(B_TOTAL, ROW)
    labels = np.ascontiguousarray(labels, dtype=np.float32).reshape(B_TOTAL, ROW)
    in_maps = []
    for c in range(NCORES):
        rows = slice(c * B_CORE, (c + 1) * B_CORE)
        in_maps.append({"pred": pred[rows], "labels": labels[rows]})
    res = run_bass_kernel_spmd(nc, in_maps, list(range(NCORES)), trace=trace)
    total = 0.0
    for c in range(NCORES):
        total += float(res.results[c]["out"][:, :3].astype(np.float64).sum())
    loss = np.float32(total / B_TOTAL)
    return loss, res


def kernel(pred, labels):
    loss, _ = run_device(pred, labels, trace=False)
    return np.array(loss, dtype=np.float32)


if __name__ == "__main__":
    rng = np.random.default_rng(0)
    p = rng.random((B_TOTAL, C, 7, 7), dtype=np.float32)
    l = rng.random((B_TOTAL, C, 7, 7), dtype=np.float32)
    l[:, 4] = (rng.random((B_TOTAL, 7, 7)) < 0.3).astype(np.float32)
    print(kernel(p, l))


# revision 21
# speedup vs baseline: 1.0974x; 1.0943x over previous
"""YOLOv1 loss kernel for Trainium2, 8-core data-parallel.

Strategy: shard batch (8192) across 8 cores (1024 each). Each core
processes its shard in NCHUNK chunks of G*128 batch items laid out as
[128 partitions, G, 1470] in SBUF (channel-major free dim). All box/IoU
arithmetic runs on the Vector engine (fused scalar_tensor_tensor ops
where possible); sqrt/square run on the Scalar (ACT) engine; DMAs on the
Sync (HWDGE) engine. Per-partition partial sums accumulate on-chip via
tensor_tensor_reduce; host sums the 8x128x3 partials and divides by B.

IoU is computed in cell-relative scaled coordinates: all three boxes of
a cell share the same (+m, +n)/G offset, so IoU is invariant to it, and
invariant to a uniform x-scale. With half-extents h = 3.5*w the overlap
width is min(2*ha, 2*hb, ha+hb-|dcx|) clamped at 0 (units: 7*w), and
union = 49*(wa*ha_frac...) i.e. 49*(w_a*h_a + w_g*h_g) - inter.
"""

import sys

import numpy as np

for _p in ("/opt/trn_rl_repo", "/root/.axon_site/_ro/trn_rl_repo"):
    if _p not in sys.path:
        sys.path.insert(0, _p)

import concourse.bass as bass
import concourse.mybir as mybir
from concourse.bass_utils import run_bass_kernel_spmd

F32 = mybir.dt.float32
U32 = mybir.dt.uint32
Alu = mybir.AluOpType
Act = mybir.ActivationFunctionType

B_TOTAL = 8192
NCORES = 8
B_CORE = B_TOTAL // NCORES  # 1024
P = 128
G = 2  # batch groups folded into the free dim per chunk
CHUNK = P * G  # 256
NCHUNK = B_CORE // CHUNK  # 4
C = 30
CELLS = 49
ROW = C * CELLS  # 1470


def build_nc(g: int = G, nchunk: int = NCHUNK):
    chunk = P * g
    nc = bass.Bass()
    pred = nc.declare_dram_parameter("pred", [B_CORE, ROW], F32, isOutput=False)
    labels = nc.declare_dram_parameter("labels", [B_CORE, ROW], F32, isOutput=False)
    out = nc.declare_dram_parameter("out", [P, 4], F32, isOutput=True)

    fshape = [P, g, CELLS]
    bshape = [P, g, 20 * CELLS]

    _ctr = [0]

    def sb(shape):
        _ctr[0] += 1
        return ctx_stack.enter_context(
            nc.sbuf_tensor(f"t{_ctr[0]}", shape, F32)
        )

    from contextlib import ExitStack

    ctx_stack = ExitStack()
    with ctx_stack:
        # double-buffered input tiles
        pt = [sb([P, g, ROW]) for _ in range(2)]
        lt = [sb([P, g, ROW]) for _ in range(2)]
        # ACT outputs (single-buffered; protected by sem schedule)
        sp2, sl2, sp3, sl3 = sb(fshape), sb(fshape), sb(fshape), sb(fshape)
        sp7, sl7, sp8, sl8 = sb(fshape), sb(fshape), sb(fshape), sb(fshape)
        q4, q9 = sb(fshape), sb(fshape)
        qx1, qy1, qx2, qy2 = sb(fshape), sb(fshape), sb(fshape), sb(fshape)
        e1, e2 = sb(fshape), sb(fshape)
        qsw1, qsh1, qsw2, qsh2 = sb(fshape), sb(fshape), sb(fshape), sb(fshape)
        sqcls = sb(bshape)
        # DVE temps
        dx1, dy1, dx2, dy2 = sb(fshape), sb(fshape), sb(fshape), sb(fshape)
        dx2c, dy2c = sb(fshape), sb(fshape)
        adx1, ady1, adx2, ady2 = sb(fshape), sb(fshape), sb(fshape), sb(fshape)
        sw1, sh1, sw2, sh2 = sb(fshape), sb(fshape), sb(fshape), sb(fshape)
        ta1, tb1, tc1 = sb(fshape), sb(fshape), sb(fshape)
        ta2, tb2, tc2 = sb(fshape), sb(fshape), sb(fshape)
        ta3, tb3, tc3 = sb(fshape), sb(fshape), sb(fshape)
        ta4, tb4, tc4 = sb(fshape), sb(fshape), sb(fshape)
        iw1, ih1, iw2, ih2 = sb(fshape), sb(fshape), sb(fshape), sb(fshape)
        int1, int2 = sb(fshape), sb(fshape)
        a1, a2, ag = sb(fshape), sb(fshape), sb(fshape)
        s1, s2 = sb(fshape), sb(fshape)
        u1, u2 = sb(fshape), sb(fshape)
        r1, r2 = sb(fshape), sb(fshape)
        iou1, iou2 = sb(fshape), sb(fshape)
        use1, objm = sb(fshape), sb(fshape)
        d1, d2 = sb(fshape), sb(fshape)
        dcls = sb(bshape)
        jbig = sb(bshape)
        dsw1, dsh1, dsw2, dsh2 = sb(fshape), sb(fshape), sb(fshape), sb(fshape)
        c1a, c1b, c2a, c2b = sb(fshape), sb(fshape), sb(fshape), sb(fshape)
        coor1, coor2 = sb(fshape), sb(fshape)
        de, hde, dc = sb(fshape), sb(fshape), sb(fshape)
        nc2t, mix, tsel = sb(fshape), sb(fshape), sb(fshape)
        clsum = sb(fshape)
        base, base2, base3, dd = sb(fshape), sb(fshape), sb(fshape), sb(fshape)
        junk = sb(fshape)
        junk2 = sb(fshape)
        red0 = ctx_stack.enter_context(nc.sbuf_tensor("red0", [P, 1], F32))
        red1 = ctx_stack.enter_context(nc.sbuf_tensor("red1", [P, 1], F32))
        acc = ctx_stack.enter_context(nc.sbuf_tensor("acc", [P, 4], F32))

        dma_sem = ctx_stack.enter_context(nc.semaphore("dma_sem"))
        sA1 = ctx_stack.enter_context(nc.semaphore("sA1"))
        sA2 = ctx_stack.enter_context(nc.semaphore("sA2"))
        sD1 = ctx_stack.enter_context(nc.semaphore("sD1"))
        sD2 = ctx_stack.enter_context(nc.semaphore("sD2"))
        v_done = ctx_stack.enter_context(nc.semaphore("v_done"))
        sGP = ctx_stack.enter_context(nc.semaphore("sGP"))
        block = ctx_stack.enter_context(nc.Block())

        def ch(t, c):  # channel slice -> [P, g, 49]
            return t[:, :, c * CELLS:(c + 1) * CELLS]

        def cls_blk(t):  # channels 10..29 -> [P, g, 980]
            return t[:, :, 10 * CELLS:30 * CELLS]

        @block.sync
        def _(sync):
            for i in range(nchunk):
                s = i % 2
                if i >= 1:
                    sync.wait_ge(dma_sem, 32 * i)
                if i >= 2:
                    sync.wait_ge(v_done, i - 1)
                rows = slice(i * chunk, (i + 1) * chunk)
                sync.dma_start(
                    out=pt[s][:],
                    in_=pred[rows].rearrange("(g p) d -> p g d", p=P),
                ).then_inc(dma_sem, 16)
                sync.dma_start(
                    out=lt[s][:],
                    in_=labels[rows].rearrange("(g p) d -> p g d", p=P),
                ).then_inc(dma_sem, 16)
            sync.wait_ge(v_done, nchunk)
            sync.dma_start(out=out[:], in_=acc[:]).then_inc(dma_sem, 16)
            sync.wait_ge(dma_sem, 32 * nchunk + 16)

        @block.gpsimd
        def _(gp):
            for i in range(nchunk):
                s = i % 2
                if i >= 1:
                    gp.wait_ge(sA2, i)
                gp.wait_ge(dma_sem, 32 * (i + 1))
                p, l = pt[s], lt[s]
                gp.tensor_tensor(dcls[:], cls_blk(p), cls_blk(l), Alu.subtract)
                gp.drain().then_inc(sGP, 1)

        @block.scalar
        def _(act):
            for i in range(nchunk):
                s = i % 2
                if i >= 1:
                    act.wait_ge(v_done, i)
                act.wait_ge(dma_sem, 32 * (i + 1))
                p, l = pt[s], lt[s]
                # phase 1: sqrts of w/h channels + conf squares
                act.activation(sp2[:], ch(p, 2), Act.Sqrt)
                act.activation(sl2[:], ch(l, 2), Act.Sqrt)
                act.activation(sp3[:], ch(p, 3), Act.Sqrt)
                act.activation(sl3[:], ch(l, 3), Act.Sqrt)
                act.activation(sp7[:], ch(p, 7), Act.Sqrt)
                act.activation(sl7[:], ch(l, 7), Act.Sqrt)
                act.activation(sp8[:], ch(p, 8), Act.Sqrt)
                act.activation(sl8[:], ch(l, 8), Act.Sqrt)
                act.activation(q4[:], ch(p, 4), Act.Square)
                act.activation(q9[:], ch(p, 9), Act.Square)
                act.drain().then_inc(sA1, 1)
                # phase 2a: squares of DVE diffs
                act.wait_ge(sD1, i + 1)
                act.activation(qx1[:], dx1[:], Act.Square)
                act.activation(qy1[:], dy1[:], Act.Square)
                act.activation(qx2[:], dx2c[:], Act.Square)
                act.activation(qy2[:], dy2c[:], Act.Square)
                act.activation(e1[:], d1[:], Act.Square)
                act.activation(e2[:], d2[:], Act.Square)
                act.wait_ge(sGP, i + 1)
                act.activation(sqcls[:], dcls[:], Act.Square)
                # phase 2b: squares of sqrt diffs
                act.wait_ge(sD2, i + 1)
                act.activation(qsw1[:], dsw1[:], Act.Square)
                act.activation(qsh1[:], dsh1[:], Act.Square)
                act.activation(qsw2[:], dsw2[:], Act.Square)
                act.activation(qsh2[:], dsh2[:], Act.Square)
                act.drain().then_inc(sA2, 1)

        @block.vector
        def _(v):
            stt = v.scalar_tensor_tensor
            tt = v.tensor_tensor
            ts = v.tensor_scalar

            v.memset(acc[:], 0.0)
            v.drain()
            for i in range(nchunk):
                s = i % 2
                v.wait_ge(dma_sem, 32 * (i + 1))
                p, l = pt[s], lt[s]
                # --- wave 1: direct from inputs ---
                tt(dx1[:], ch(p, 0), ch(l, 0), Alu.subtract)
                tt(dy1[:], ch(p, 1), ch(l, 1), Alu.subtract)
                tt(dx2[:], ch(p, 5), ch(l, 0), Alu.subtract)
                tt(dy2[:], ch(p, 6), ch(l, 1), Alu.subtract)
                tt(dx2c[:], ch(p, 5), ch(l, 5), Alu.subtract)
                tt(dy2c[:], ch(p, 6), ch(l, 6), Alu.subtract)
                tt(sw1[:], ch(p, 2), ch(l, 2), Alu.add)
                tt(sh1[:], ch(p, 3), ch(l, 3), Alu.add)
                tt(sw2[:], ch(p, 7), ch(l, 2), Alu.add)
                tt(sh2[:], ch(p, 8), ch(l, 3), Alu.add)
                tt(tc1[:], ch(p, 2), ch(l, 2), Alu.min)
                tt(tc2[:], ch(p, 3), ch(l, 3), Alu.min)
                tt(tc3[:], ch(p, 7), ch(l, 2), Alu.min)
                tt(tc4[:], ch(p, 8), ch(l, 3), Alu.min)
                tt(a1[:], ch(p, 2), ch(p, 3), Alu.mult)
                tt(a2[:], ch(p, 7), ch(p, 8), Alu.mult)
                tt(ag[:], ch(l, 2), ch(l, 3), Alu.mult)
                ts(objm[:], ch(l, 4), 1.0, None, Alu.is_equal)
                v.drain()
                # --- wave 2 ---
                ts(adx1[:].bitcast(U32), dx1[:].bitcast(U32), 0x7FFFFFFF, None,
                   Alu.bitwise_and)
                ts(ady1[:].bitcast(U32), dy1[:].bitcast(U32), 0x7FFFFFFF, None,
                   Alu.bitwise_and)
                ts(adx2[:].bitcast(U32), dx2[:].bitcast(U32), 0x7FFFFFFF, None,
                   Alu.bitwise_and)
                ts(ady2[:].bitcast(U32), dy2[:].bitcast(U32), 0x7FFFFFFF, None,
                   Alu.bitwise_and)
                tt(s1[:], a1[:], ag[:], Alu.add)
                tt(s2[:], a2[:], ag[:], Alu.add)
                v.drain()
                # --- wave 3: overlap = min(S-|d|, 7wa, 7wb), clamped ---
                stt(ta1[:], sw1[:], 3.5, adx1[:], Alu.mult, Alu.subtract)
                stt(ta2[:], sh1[:], 3.5, ady1[:], Alu.mult, Alu.subtract)
                stt(ta3[:], sw2[:], 3.5, adx2[:], Alu.mult, Alu.subtract)
                stt(ta4[:], sh2[:], 3.5, ady2[:], Alu.mult, Alu.subtract)
                v.drain()
                # --- wave 4: min vs 7*min(wa,wb) ---
                stt(tb1[:], tc1[:], 7.0, ta1[:], Alu.mult, Alu.min)
                stt(tb2[:], tc2[:], 7.0, ta2[:], Alu.mult, Alu.min)
                stt(tb3[:], tc3[:], 7.0, ta3[:], Alu.mult, Alu.min)
                stt(tb4[:], tc4[:], 7.0, ta4[:], Alu.mult, Alu.min)
                v.drain()
                # --- wave 5: clamp ---
                ts(iw1[:], tb1[:], 0.0, None, Alu.max)
                ts(ih1[:], tb2[:], 0.0, None, Alu.max)
                ts(iw2[:], tb3[:], 0.0, None, Alu.max)
                ts(ih2[:], tb4[:], 0.0, None, Alu.max)
                v.drain()
                # --- wave 7 ---
                tt(int1[:], iw1[:], ih1[:], Alu.mult)
                tt(int2[:], iw2[:], ih2[:], Alu.mult)
                v.drain()
                # --- wave 8: union = 49*(area_p + area_g) - inter ---
                stt(u1[:], s1[:], 49.0, int1[:], Alu.mult, Alu.subtract)
                stt(u2[:], s2[:], 49.0, int2[:], Alu.mult, Alu.subtract)
                v.drain()
                # --- wave 9 ---
                v.reciprocal(r1[:], u1[:])
                v.reciprocal(r2[:], u2[:])
                v.drain()
                # --- wave 10 ---
                tt(iou1[:], int1[:], r1[:], Alu.mult)
                tt(iou2[:], int2[:], r2[:], Alu.mult)
                v.drain()
                # --- wave 11 ---
                tt(use1[:], iou1[:], iou2[:], Alu.is_ge)
                tt(d1[:], ch(p, 4), iou1[:], Alu.subtract)
                tt(d2[:], ch(p, 9), iou2[:], Alu.subtract)
                v.drain().then_inc(sD1, 1)
                # --- wave 12: sqrt diffs (needs ACT phase 1) ---
                v.wait_ge(sA1, i + 1)
                tt(dsw1[:], sp2[:], sl2[:], Alu.subtract)
                tt(dsh1[:], sp3[:], sl3[:], Alu.subtract)
                tt(dsw2[:], sp7[:], sl7[:], Alu.subtract)
                tt(dsh2[:], sp8[:], sl8[:], Alu.subtract)
                v.drain().then_inc(sD2, 1)
                # --- wave 13+: combine (needs ACT phase 2) ---
                v.wait_ge(sA2, i + 1)
                v.tensor_reduce(
                    out=clsum[:],
                    in_=sqcls[:].rearrange("p g (c k) -> p g k c", c=20),
                    axis=mybir.AxisListType.X, op=Alu.add,
                )
                tt(de[:], e1[:], e2[:], Alu.subtract)
                tt(nc2t[:], q4[:], q9[:], Alu.add)
                tt(c1a[:], qx1[:], qy1[:], Alu.add)
                tt(c1b[:], qsw1[:], qsh1[:], Alu.add)
                tt(c2a[:], qx2[:], qy2[:], Alu.add)
                tt(c2b[:], qsw2[:], qsh2[:], Alu.add)
                v.drain()
                tt(coor1[:], c1a[:], c1b[:], Alu.add)
                tt(coor2[:], c2a[:], c2b[:], Alu.add)
                ts(hde[:], de[:], 0.5, None, Alu.mult)
                v.drain()
                tt(dc[:], coor1[:], coor2[:], Alu.subtract)
                stt(base[:], coor2[:], 5.0, e2[:], Alu.mult, Alu.add)
                v.drain()
                stt(mix[:], dc[:], 5.0, hde[:], Alu.mult, Alu.add)
                stt(base2[:], e1[:], 0.5, base[:], Alu.mult, Alu.add)
                v.drain()
                tt(tsel[:], use1[:], mix[:], Alu.mult)
                tt(junk2[:], clsum[:], base2[:], Alu.add)
                v.drain()
                tt(base3[:], junk2[:], tsel[:], Alu.add)
                v.drain()
                stt(dd[:], nc2t[:], -0.5, base3[:], Alu.mult, Alu.add)
                v.drain()
                # accumulate: acc0 += sum(obj * dd); acc1 += 0.5*sum(nc2)
                tt(junk[:], objm[:], dd[:], Alu.mult)
                v.drain()
                v.tensor_reduce(out=red0[:], in_=junk[:],
                                axis=mybir.AxisListType.XY, op=Alu.add)
                v.tensor_reduce(out=red1[:], in_=nc2t[:],
                                axis=mybir.AxisListType.XY, op=Alu.add)
                v.drain()
                stt(acc[:, 0:1], red0[:], 1.0, acc[:, 0:1], Alu.mult, Alu.add)
                stt(acc[:, 1:2], red1[:], 0.5, acc[:, 1:2], Alu.mult, Alu.add)
                v.drain().then_inc(v_done, 1)

    return nc


_NC_CACHE = {}


def _get_nc():
    if "nc" not in _NC_CACHE:
        _NC_CACHE["nc"] = build_nc()
    return _NC_CACHE["nc"]


def run_device(pred, labels, trace=False):
    nc = _get_nc()
    pred = np.ascontiguousarray(pred, dtype=np.float32).reshape(B_TOTAL, ROW)
    labels = np.ascontiguousarray(labels, dtype=np.float32).reshape(B_TOTAL, ROW)
    in_maps = []
    for c in range(NCORES):
        rows = slice(c * B_CORE, (c + 1) * B_CORE)
        in_maps.append({"pred": pred[rows], "labels": labels[rows]})
    res = run_bass_kernel_spmd(nc, in_maps, list(range(NCORES)), trace=trace)
    total = 0.0
    for c in range(NCORES):
        total += float(res.results[c]["out"][:, :3].astype(np.float64).sum())
    loss = np.float32(total / B_TOTAL)
    return loss, res


def kernel(pred, labels):
    loss, _ = run_device(pred, labels, trace=False)
    return np.array(loss, dtype=np.float32)


if __name__ == "__main__":
    rng = np.random.default_rng(0)
    p = rng.random((B_TOTAL, C, 7, 7), dtype=np.float32)
    l = rng.random((B_TOTAL, C, 7, 7), dtype=np.float32)
    l[:, 4] = (rng.random((B_TOTAL, 7, 7)) < 0.3).astype(np.float32)
    print(kernel(p, l))


# revision 22
# speedup vs baseline: 1.1194x; 1.0200x over previous
"""YOLOv1 loss kernel for Trainium2, 8-core data-parallel.

Strategy: shard batch (8192) across 8 cores (1024 each). Each core
processes its shard in NCHUNK chunks of G*128 batch items laid out as
[128 partitions, G, 1470] in SBUF (channel-major free dim). All box/IoU
arithmetic runs on the Vector engine (fused scalar_tensor_tensor ops
where possible); sqrt/square run on the Scalar (ACT) engine; DMAs on the
Sync (HWDGE) engine. Per-partition partial sums accumulate on-chip via
tensor_tensor_reduce; host sums the 8x128x3 partials and divides by B.

IoU is computed in cell-relative scaled coordinates: all three boxes of
a cell share the same (+m, +n)/G offset, so IoU is invariant to it, and
invariant to a uniform x-scale. With half-extents h = 3.5*w the overlap
width is min(2*ha, 2*hb, ha+hb-|dcx|) clamped at 0 (units: 7*w), and
union = 49*(wa*ha_frac...) i.e. 49*(w_a*h_a + w_g*h_g) - inter.
"""

import sys

import numpy as np

for _p in ("/opt/trn_rl_repo", "/root/.axon_site/_ro/trn_rl_repo"):
    if _p not in sys.path:
        sys.path.insert(0, _p)

import concourse.bass as bass
import concourse.mybir as mybir
from concourse.bass_utils import run_bass_kernel_spmd

F32 = mybir.dt.float32
U32 = mybir.dt.uint32
Alu = mybir.AluOpType
Act = mybir.ActivationFunctionType

B_TOTAL = 8192
NCORES = 8
B_CORE = B_TOTAL // NCORES  # 1024
P = 128
G = 2  # batch groups folded into the free dim per chunk
CHUNK = P * G  # 256
NCHUNK = B_CORE // CHUNK  # 4
C = 30
CELLS = 49
ROW = C * CELLS  # 1470


def build_nc(g: int = G, nchunk: int = NCHUNK):
    chunk = P * g
    nc = bass.Bass()
    pred = nc.declare_dram_parameter("pred", [B_CORE, ROW], F32, isOutput=False)
    labels = nc.declare_dram_parameter("labels", [B_CORE, ROW], F32, isOutput=False)
    out = nc.declare_dram_parameter("out", [P, 4], F32, isOutput=True)

    fshape = [P, g, CELLS]
    bshape = [P, g, 20 * CELLS]

    _ctr = [0]

    def sb(shape):
        _ctr[0] += 1
        return ctx_stack.enter_context(
            nc.sbuf_tensor(f"t{_ctr[0]}", shape, F32)
        )

    from contextlib import ExitStack

    ctx_stack = ExitStack()
    with ctx_stack:
        # double-buffered input tiles
        pt = [sb([P, g, ROW]) for _ in range(2)]
        lt = [sb([P, g, ROW]) for _ in range(2)]
        # ACT outputs (single-buffered; protected by sem schedule)
        sp2, sl2, sp3, sl3 = sb(fshape), sb(fshape), sb(fshape), sb(fshape)
        sp7, sl7, sp8, sl8 = sb(fshape), sb(fshape), sb(fshape), sb(fshape)
        q4, q9 = sb(fshape), sb(fshape)
        qx1, qy1, qx2, qy2 = sb(fshape), sb(fshape), sb(fshape), sb(fshape)
        e1, e2 = sb(fshape), sb(fshape)
        qsw1, qsh1, qsw2, qsh2 = sb(fshape), sb(fshape), sb(fshape), sb(fshape)
        sqcls = sb(bshape)
        # DVE temps
        dx1, dy1, dx2, dy2 = sb(fshape), sb(fshape), sb(fshape), sb(fshape)
        dx2c, dy2c = sb(fshape), sb(fshape)
        adx1, ady1, adx2, ady2 = sb(fshape), sb(fshape), sb(fshape), sb(fshape)
        sw1, sh1, sw2, sh2 = sb(fshape), sb(fshape), sb(fshape), sb(fshape)
        ta1, tb1, tc1 = sb(fshape), sb(fshape), sb(fshape)
        ta2, tb2, tc2 = sb(fshape), sb(fshape), sb(fshape)
        ta3, tb3, tc3 = sb(fshape), sb(fshape), sb(fshape)
        ta4, tb4, tc4 = sb(fshape), sb(fshape), sb(fshape)
        iw1, ih1, iw2, ih2 = sb(fshape), sb(fshape), sb(fshape), sb(fshape)
        int1, int2 = sb(fshape), sb(fshape)
        a1, a2, ag = sb(fshape), sb(fshape), sb(fshape)
        s1, s2 = sb(fshape), sb(fshape)
        u1, u2 = sb(fshape), sb(fshape)
        r1, r2 = sb(fshape), sb(fshape)
        iou1, iou2 = sb(fshape), sb(fshape)
        use1, objm = sb(fshape), sb(fshape)
        d1, d2 = sb(fshape), sb(fshape)
        dcls = sb(bshape)
        jbig = sb(bshape)
        dsw1, dsh1, dsw2, dsh2 = sb(fshape), sb(fshape), sb(fshape), sb(fshape)
        c1a, c1b, c2a, c2b = sb(fshape), sb(fshape), sb(fshape), sb(fshape)
        coor1, coor2 = sb(fshape), sb(fshape)
        de, hde, dc = sb(fshape), sb(fshape), sb(fshape)
        nc2t, mix, tsel = sb(fshape), sb(fshape), sb(fshape)
        clsum = sb(fshape)
        base, base2, base3, dd = sb(fshape), sb(fshape), sb(fshape), sb(fshape)
        junk = sb(fshape)
        junk2 = sb(fshape)
        red0 = ctx_stack.enter_context(nc.sbuf_tensor("red0", [P, 1], F32))
        red1 = ctx_stack.enter_context(nc.sbuf_tensor("red1", [P, 1], F32))
        acc = ctx_stack.enter_context(nc.sbuf_tensor("acc", [P, 4], F32))

        dma_sem = ctx_stack.enter_context(nc.semaphore("dma_sem"))
        sA1 = ctx_stack.enter_context(nc.semaphore("sA1"))
        sA2 = ctx_stack.enter_context(nc.semaphore("sA2"))
        sD1 = ctx_stack.enter_context(nc.semaphore("sD1"))
        sD2 = ctx_stack.enter_context(nc.semaphore("sD2"))
        v_done = ctx_stack.enter_context(nc.semaphore("v_done"))
        sGP = ctx_stack.enter_context(nc.semaphore("sGP"))
        block = ctx_stack.enter_context(nc.Block())

        def ch(t, c):  # channel slice -> [P, g, 49]
            return t[:, :, c * CELLS:(c + 1) * CELLS]

        def cls_blk(t):  # channels 10..29 -> [P, g, 980]
            return t[:, :, 10 * CELLS:30 * CELLS]

        @block.sync
        def _(sync):
            for i in range(nchunk):
                s = i % 2
                if i >= 1:
                    sync.wait_ge(dma_sem, 32 * i)
                if i >= 2:
                    sync.wait_ge(v_done, i - 1)
                rows = slice(i * chunk, (i + 1) * chunk)
                sync.dma_start(
                    out=pt[s][:],
                    in_=pred[rows].rearrange("(g p) d -> p g d", p=P),
                ).then_inc(dma_sem, 16)
                sync.dma_start(
                    out=lt[s][:],
                    in_=labels[rows].rearrange("(g p) d -> p g d", p=P),
                ).then_inc(dma_sem, 16)
            sync.wait_ge(v_done, nchunk)
            sync.dma_start(out=out[:], in_=acc[:]).then_inc(dma_sem, 16)
            sync.wait_ge(dma_sem, 32 * nchunk + 16)

        @block.gpsimd
        def _(gp):
            for i in range(nchunk):
                s = i % 2
                if i >= 1:
                    gp.wait_ge(v_done, i)
                gp.wait_ge(dma_sem, 32 * (i + 1))
                p, l = pt[s], lt[s]
                gp.tensor_tensor(dx2c[:], ch(p, 5), ch(l, 5), Alu.subtract)
                gp.tensor_tensor(dy2c[:], ch(p, 6), ch(l, 6), Alu.subtract)
                gp.tensor_scalar(objm[:], ch(l, 4), 1.0, None, Alu.is_equal)
                gp.tensor_tensor(dcls[:], cls_blk(p), cls_blk(l), Alu.subtract)
                gp.drain().then_inc(sGP, 1)

        @block.scalar
        def _(act):
            for i in range(nchunk):
                s = i % 2
                if i >= 1:
                    act.wait_ge(v_done, i)
                act.wait_ge(dma_sem, 32 * (i + 1))
                p, l = pt[s], lt[s]
                # phase 1: sqrts of w/h channels + conf squares
                act.activation(sp2[:], ch(p, 2), Act.Sqrt)
                act.activation(sl2[:], ch(l, 2), Act.Sqrt)
                act.activation(sp3[:], ch(p, 3), Act.Sqrt)
                act.activation(sl3[:], ch(l, 3), Act.Sqrt)
                act.activation(sp7[:], ch(p, 7), Act.Sqrt)
                act.activation(sl7[:], ch(l, 7), Act.Sqrt)
                act.activation(sp8[:], ch(p, 8), Act.Sqrt)
                act.activation(sl8[:], ch(l, 8), Act.Sqrt)
                act.activation(q4[:], ch(p, 4), Act.Square)
                act.activation(q9[:], ch(p, 9), Act.Square)
                act.drain().then_inc(sA1, 1)
                # phase 2a: squares of DVE diffs
                act.wait_ge(sD1, i + 1)
                act.activation(qx1[:], dx1[:], Act.Square)
                act.activation(qy1[:], dy1[:], Act.Square)
                act.activation(e1[:], d1[:], Act.Square)
                act.activation(e2[:], d2[:], Act.Square)
                act.wait_ge(sGP, i + 1)
                act.activation(qx2[:], dx2c[:], Act.Square)
                act.activation(qy2[:], dy2c[:], Act.Square)
                act.activation(sqcls[:], dcls[:], Act.Square)
                # phase 2b: squares of sqrt diffs
                act.wait_ge(sD2, i + 1)
                act.activation(qsw1[:], dsw1[:], Act.Square)
                act.activation(qsh1[:], dsh1[:], Act.Square)
                act.activation(qsw2[:], dsw2[:], Act.Square)
                act.activation(qsh2[:], dsh2[:], Act.Square)
                act.drain().then_inc(sA2, 1)

        @block.vector
        def _(v):
            stt = v.scalar_tensor_tensor
            tt = v.tensor_tensor
            ts = v.tensor_scalar

            v.memset(acc[:], 0.0)
            v.drain()
            for i in range(nchunk):
                s = i % 2
                v.wait_ge(dma_sem, 32 * (i + 1))
                p, l = pt[s], lt[s]
                # --- wave 1: direct from inputs ---
                tt(dx1[:], ch(p, 0), ch(l, 0), Alu.subtract)
                tt(dy1[:], ch(p, 1), ch(l, 1), Alu.subtract)
                tt(dx2[:], ch(p, 5), ch(l, 0), Alu.subtract)
                tt(dy2[:], ch(p, 6), ch(l, 1), Alu.subtract)
                tt(sw1[:], ch(p, 2), ch(l, 2), Alu.add)
                tt(sh1[:], ch(p, 3), ch(l, 3), Alu.add)
                tt(sw2[:], ch(p, 7), ch(l, 2), Alu.add)
                tt(sh2[:], ch(p, 8), ch(l, 3), Alu.add)
                tt(tc1[:], ch(p, 2), ch(l, 2), Alu.min)
                tt(tc2[:], ch(p, 3), ch(l, 3), Alu.min)
                tt(tc3[:], ch(p, 7), ch(l, 2), Alu.min)
                tt(tc4[:], ch(p, 8), ch(l, 3), Alu.min)
                tt(a1[:], ch(p, 2), ch(p, 3), Alu.mult)
                tt(a2[:], ch(p, 7), ch(p, 8), Alu.mult)
                tt(ag[:], ch(l, 2), ch(l, 3), Alu.mult)
                v.drain()
                # --- wave 2 ---
                ts(adx1[:].bitcast(U32), dx1[:].bitcast(U32), 0x7FFFFFFF, None,
                   Alu.bitwise_and)
                ts(ady1[:].bitcast(U32), dy1[:].bitcast(U32), 0x7FFFFFFF, None,
                   Alu.bitwise_and)
                ts(adx2[:].bitcast(U32), dx2[:].bitcast(U32), 0x7FFFFFFF, None,
                   Alu.bitwise_and)
                ts(ady2[:].bitcast(U32), dy2[:].bitcast(U32), 0x7FFFFFFF, None,
                   Alu.bitwise_and)
                tt(s1[:], a1[:], ag[:], Alu.add)
                tt(s2[:], a2[:], ag[:], Alu.add)
                v.drain()
                # --- wave 3: overlap = min(S-|d|, 7wa, 7wb), clamped ---
                stt(ta1[:], sw1[:], 3.5, adx1[:], Alu.mult, Alu.subtract)
                stt(ta2[:], sh1[:], 3.5, ady1[:], Alu.mult, Alu.subtract)
                stt(ta3[:], sw2[:], 3.5, adx2[:], Alu.mult, Alu.subtract)
                stt(ta4[:], sh2[:], 3.5, ady2[:], Alu.mult, Alu.subtract)
                v.drain()
                # --- wave 4: min vs 7*min(wa,wb) ---
                stt(tb1[:], tc1[:], 7.0, ta1[:], Alu.mult, Alu.min)
                stt(tb2[:], tc2[:], 7.0, ta2[:], Alu.mult, Alu.min)
                stt(tb3[:], tc3[:], 7.0, ta3[:], Alu.mult, Alu.min)
                stt(tb4[:], tc4[:], 7.0, ta4[:], Alu.mult, Alu.min)
                v.drain()
                # --- wave 5: clamp ---
                ts(iw1[:], tb1[:], 0.0, None, Alu.max)
                ts(ih1[:], tb2[:], 0.0, None, Alu.max)
                ts(iw2[:], tb3[:], 0.0, None, Alu.max)
                ts(ih2[:], tb4[:], 0.0, None, Alu.max)
                v.drain()
                # --- wave 7 ---
                tt(int1[:], iw1[:], ih1[:], Alu.mult)
                tt(int2[:], iw2[:], ih2[:], Alu.mult)
                v.drain()
                # --- wave 8: union = 49*(area_p + area_g) - inter ---
                stt(u1[:], s1[:], 49.0, int1[:], Alu.mult, Alu.subtract)
                stt(u2[:], s2[:], 49.0, int2[:], Alu.mult, Alu.subtract)
                v.drain()
                # --- wave 9 ---
                v.reciprocal(r1[:], u1[:])
                v.reciprocal(r2[:], u2[:])
                v.drain()
                # --- wave 10 ---
                tt(iou1[:], int1[:], r1[:], Alu.mult)
                tt(iou2[:], int2[:], r2[:], Alu.mult)
                v.drain()
                # --- wave 11 ---
                tt(use1[:], iou1[:], iou2[:], Alu.is_ge)
                tt(d1[:], ch(p, 4), iou1[:], Alu.subtract)
                tt(d2[:], ch(p, 9), iou2[:], Alu.subtract)
                v.drain().then_inc(sD1, 1)
                # --- wave 12: sqrt diffs (needs ACT phase 1) ---
                v.wait_ge(sA1, i + 1)
                tt(dsw1[:], sp2[:], sl2[:], Alu.subtract)
                tt(dsh1[:], sp3[:], sl3[:], Alu.subtract)
                tt(dsw2[:], sp7[:], sl7[:], Alu.subtract)
                tt(dsh2[:], sp8[:], sl8[:], Alu.subtract)
                v.drain().then_inc(sD2, 1)
                # --- wave 13+: combine (needs ACT phase 2) ---
                v.wait_ge(sA2, i + 1)
                v.tensor_reduce(
                    out=clsum[:],
                    in_=sqcls[:].rearrange("p g (c k) -> p g k c", c=20),
                    axis=mybir.AxisListType.X, op=Alu.add,
                )
                tt(de[:], e1[:], e2[:], Alu.subtract)
                tt(nc2t[:], q4[:], q9[:], Alu.add)
                tt(c1a[:], qx1[:], qy1[:], Alu.add)
                tt(c1b[:], qsw1[:], qsh1[:], Alu.add)
                tt(c2a[:], qx2[:], qy2[:], Alu.add)
                tt(c2b[:], qsw2[:], qsh2[:], Alu.add)
                v.drain()
                tt(coor1[:], c1a[:], c1b[:], Alu.add)
                tt(coor2[:], c2a[:], c2b[:], Alu.add)
                ts(hde[:], de[:], 0.5, None, Alu.mult)
                v.drain()
                tt(dc[:], coor1[:], coor2[:], Alu.subtract)
                stt(base[:], coor2[:], 5.0, e2[:], Alu.mult, Alu.add)
                v.drain()
                stt(mix[:], dc[:], 5.0, hde[:], Alu.mult, Alu.add)
                stt(base2[:], e1[:], 0.5, base[:], Alu.mult, Alu.add)
                v.drain()
                tt(tsel[:], use1[:], mix[:], Alu.mult)
                tt(junk2[:], clsum[:], base2[:], Alu.add)
                v.drain()
                tt(base3[:], junk2[:], tsel[:], Alu.add)
                v.drain()
                stt(dd[:], nc2t[:], -0.5, base3[:], Alu.mult, Alu.add)
                v.drain()
                # accumulate: acc0 += sum(obj * dd); acc1 += 0.5*sum(nc2)
                tt(junk[:], objm[:], dd[:], Alu.mult)
                v.drain()
                v.tensor_reduce(out=red0[:], in_=junk[:],
                                axis=mybir.AxisListType.XY, op=Alu.add)
                v.tensor_reduce(out=red1[:], in_=nc2t[:],
                                axis=mybir.AxisListType.XY, op=Alu.add)
                v.drain()
                stt(acc[:, 0:1], red0[:], 1.0, acc[:, 0:1], Alu.mult, Alu.add)
                stt(acc[:, 1:2], red1[:], 0.5, acc[:, 1:2], Alu.mult, Alu.add)
                v.drain().then_inc(v_done, 1)

    return nc


_NC_CACHE = {}


def _get_nc():
    if "nc" not in _NC_CACHE:
        _NC_CACHE["nc"] = build_nc()
    return _NC_CACHE["nc"]


def run_device(pred, labels, trace=False):
    nc = _get_nc()
    pred = np.ascontiguousarray(pred, dtype=np.float32).reshape(B_TOTAL, ROW)
    labels = np.ascontiguousarray(labels, dtype=np.float32).reshape(B_TOTAL, ROW)
    in_maps = []
    for c in range(NCORES):
        rows = slice(c * B_CORE, (c + 1) * B_CORE)
        in_maps.append({"pred": pred[rows], "labels": labels[rows]})
    res = run_bass_kernel_spmd(nc, in_maps, list(range(NCORES)), trace=trace)
    total = 0.0
    for c in range(NCORES):
        total += float(res.results[c]["out"][:, :3].astype(np.float64).sum())
    loss = np.float32(total / B_TOTAL)
    return loss, res


def kernel(pred, labels):
    loss, _ = run_device(pred, labels, trace=False)
    return np.array(loss, dtype=np.float32)


if __name__ == "__main__":
    rng = np.random.default_rng(0)
    p = rng.random((B_TOTAL, C, 7, 7), dtype=np.float32)
    l = rng.random((B_TOTAL, C, 7, 7), dtype=np.float32)
    l[:, 4] = (rng.random((B_TOTAL, 7, 7)) < 0.3).astype(np.float32)
    print(kernel(p, l))
